# revision 31
# baseline (speedup 1.0000x reference)
"""Trainium2 Bass kernel for a LoRA-MoE layer (gate top-2 softmax routing +
dense base linear + per-expert low-rank adapters), SPMD across 8 NeuronCores.

Math (per token t):
    logits = x @ gate_w.T                      # [E]
    top-2 softmax over logits -> dense w[E] (0 for non-selected)
    out = x @ base_w.T + base_b
        + SCALING * sum_e w[e] * (x @ lora_A[e].T) @ lora_B[e].T

Key identities:
  - with w folded into the rank-space activations,
        lora_out = (low * w_rep) @ B_all.T,  low = x @ A_all.T  (A_all: [E*R, D])
  - the two top-2 softmax weights are sigmoid(2*logit - m1 - m2) where m1/m2
    are the top-2 logit values (sigmoid(m1-m2) and sigmoid(m2-m1)).

Sharding: pure data parallel - 8 token groups of 512 tokens; every core
computes all 4096 out features for its tokens.  All matmul operands are
bf16 (PSUM accumulation stays fp32): same 1 col/cycle PE throughput as
f32r but 2x lighter DMA and 4x faster LDWEIGHTS (fast weight load).

Layout per core (contraction dim on partitions):
    out.T[o, t] = sum_d W[o, d] * x.T[d, t]    (x.T moving, W tiles stationary)

Schedule: the first out-tile's k-loop also carries the lora-low and gate
matmuls (one pass over x while x streams in from HBM); each out-tile's
lora accumulate + store is deferred behind the next tile's W matmuls so
the gating vector chain never stalls the PE.
"""

import numpy as np
import ml_dtypes

import concourse.bass as bass
import concourse.bass_isa as bass_isa
import concourse.mybir as mybir
import concourse.tile as tile
from concourse import bacc
from concourse.bass_utils import run_bass_kernel_spmd

F32 = mybir.dt.float32
BF16 = mybir.dt.bfloat16
NP_BF16 = ml_dtypes.bfloat16

# Problem constants
B, S, D, O = 2, 2048, 4096, 4096
E, R = 8, 16
ER = E * R  # 128
SCALING = 32.0 / 16.0

# Sharding: 8 token groups (pure data parallel)
N_CORES = 8
T = (B * S) // N_CORES  # 512 tokens per core
KT = D // 128           # 32 contraction tiles
OTN = O // 128          # 32 out tiles per core
WCW = 16                # W DMA chunks per out tile: 2 chunks of 16 k-tiles
# k-tile chunk boundaries for the streamed first-pass tensors (x, c, w012):
# small quanta first so the PE can start early, then 4-k quanta (each dma
# issue costs ~0.63us on the issuing queue, so quanta can't be too small)
CB = [0, 2, 4, 8, 12, 16, 20, 24, 28, 32]
NCH = len(CB) - 1
NMERGE = 3              # out-tiles folded into the first k-loop (ot0..2)


def _ci(k):
    for i in range(NCH):
        if k < CB[i + 1]:
            return i
    raise ValueError(k)


def build_body(nc, tc, tensors):
    xT, wT, wT01, cT, bT, bias2, Rm, out = tensors
    OP = mybir.AluOpType

    with (
        tc.tile_pool(name="xp", bufs=NCH) as xp,
        tc.tile_pool(name="apl", bufs=NCH) as apl,
        tc.tile_pool(name="wp", bufs=8) as wp,
        tc.tile_pool(name="wp01", bufs=NCH) as wp01,
        tc.tile_pool(name="cst", bufs=1) as cst,
        tc.tile_pool(name="gw", bufs=1) as gw,
        tc.tile_pool(name="outp", bufs=3) as outp,
        tc.tile_pool(name="psA", bufs=1, space="PSUM") as psA,
        tc.tile_pool(name="psB", bufs=6, space="PSUM") as psB,
    ):
        # ---- tiny constants on the scalar HWDGE queue (it starves behind
        # the sync stream but these are only needed tens of us in); gpsimd
        # issues no DMAs at all so the tail SWDGE drain is empty ----
        Rm_sb = cst.tile([E, ER], BF16, tag="rm")
        nc.scalar.dma_start(out=Rm_sb[:], in_=Rm[:])
        bias_sb = cst.tile([128, OTN], F32, tag="bias")
        nc.scalar.dma_start(out=bias_sb[:], in_=bias2[:])
        bT_sb = cst.tile([ER, O], BF16, tag="bt")
        nc.scalar.dma_start(out=bT_sb[:], in_=bT[:])

        # preload the ACT sigmoid table off the gating chain's critical path
        sgdum = gw.tile([1, 8], F32, tag="sgdum")
        nc.scalar.activation(sgdum[:], bias_sb[0:1, 0:8],
                             mybir.ActivationFunctionType.Sigmoid)

        # warm the PE HAM clock gate with a sustained burst of throwaway
        # matmuls on zeroed data while the first x/c/w012 quanta are still in
        # flight: the un-throttle (1.2 -> 2.4 GHz) needs ~3.4us of nearly
        # continuous PE busy, which the DMA-paced real stream cannot provide
        dum_sb = gw.tile([128, 128], BF16, tag="dum")
        nc.vector.memset(dum_sb[:], 0.0)
        dum_ps = psA.tile([128, 128], F32, tag="gatewrep")
        for _ in range(44):
            nc.tensor.matmul(dum_ps[:], lhsT=dum_sb[:], rhs=dum_sb[:],
                             start=True, stop=True)

        # ---- streaming inputs on sync, in exact consumption order.  The
        # merged loop runs 5 matmuls per k-tile (low, gate, W-ot0/1/2),
        # eating x(131K) + c(35K) + w012(98K) bf16 per 1.08us = ~245 GB/s,
        # with real slack against the ~330 GB/s a single HWDGE stream
        # delivers.  gate_w/lora_A are fused into one cT tensor and the
        # first three W tiles into one wT012 tensor so each quantum is only
        # three dma issues (~0.63us of issue time each on the sync queue) ----
        x_ch, c_ch, w012_ch = [], [], []
        w_ch = {}

        def w_dma(ot, j, eng):
            wq = wp.tile([128, WCW, 128], BF16, tag="w", name=f"w{ot}_{j}")
            eng.dma_start(out=wq[:], in_=wT[:, ot, j * WCW:(j + 1) * WCW, :])
            w_ch[(ot, j)] = wq

        for c in range(NCH):
            k0, k1 = CB[c], CB[c + 1]
            xc = xp.tile([128, k1 - k0, T], BF16, tag="x", name=f"x{c}")
            nc.sync.dma_start(out=xc[:], in_=xT[:, k0:k1, :])
            x_ch.append(xc)
            cc = apl.tile([128, k1 - k0, E + ER], BF16, tag="c", name=f"c{c}")
            nc.sync.dma_start(out=cc[:], in_=cT[:, k0:k1, :])
            c_ch.append(cc)
            wc = wp01.tile([128, k1 - k0, NMERGE * 128], BF16, tag="w012",
                           name=f"w012_{c}")
            nc.sync.dma_start(out=wc[:], in_=wT01[:, k0:k1, :])
            w012_ch.append(wc)
        w_dma(3, 0, nc.sync); w_dma(3, 1, nc.sync)
        w_dma(4, 0, nc.sync); w_dma(4, 1, nc.sync)

        def x_at(k):
            c = _ci(k)
            return x_ch[c][:, k - CB[c], :]

        def w_at(ot, k):
            if ot < NMERGE:
                c = _ci(k)
                return w012_ch[c][:, k - CB[c], ot * 128:(ot + 1) * 128]
            return w_ch[(ot, k // WCW)][:, k % WCW, :]

        # ---- merged first pass: low.T, gate.T and out-tiles 0-2 W matmuls ----
        low_ps = psA.tile([ER, T], F32, tag="low")
        gate_ps = psA.tile([E, T], F32, tag="gatewrep")
        dum2_ps = psB.tile([128, 128], F32, tag="pb", name="dumps2")
        pb = {}
        for ot in range(NMERGE):
            pb[ot] = psB.tile([128, T], F32, tag="pb", name=f"pb{ot}")
        # runs of same-accumulator matmuls per quantum: switching the PSUM
        # accumulation target costs ~90ns, so amortize it over 2-4 matmuls.
        # After the first two quanta, bridge the still-ramping DMA stream
        # with a few more dummy matmuls: a PE-idle run >3.4us re-throttles
        # the clock to 1.2GHz, which costs far more than the dummies
        for c in range(NCH):
            k0, k1 = CB[c], CB[c + 1]
            for k in range(k0, k1):
                nc.tensor.matmul(low_ps[:], lhsT=c_ch[c][:, k - k0, E:E + ER],
                                 rhs=x_at(k), start=(k == 0), stop=(k == KT - 1))
            for k in range(k0, k1):
                nc.tensor.matmul(gate_ps[:], lhsT=c_ch[c][:, k - k0, 0:E],
                                 rhs=x_at(k), start=(k == 0), stop=(k == KT - 1))
            for ot in range(NMERGE):
                for k in range(k0, k1):
                    nc.tensor.matmul(pb[ot][:], lhsT=w_at(ot, k), rhs=x_at(k),
                                     start=(k == 0), stop=False)
            if c <= 1:
                for _ in range(12):
                    nc.tensor.matmul(dum2_ps[:], lhsT=dum_sb[:], rhs=dum_sb[:],
                                     start=True, stop=True)

        # ---- gating math in [E, t] layout (DVE/ACT/GPSIMD, off PE path) ----
        g_sb = gw.tile([E, T], F32, tag="gsb")
        nc.vector.tensor_copy(g_sb[:], gate_ps[:])
        m1b = gw.tile([E, T], F32, tag="m1b")
        nc.gpsimd.partition_all_reduce(m1b[:], g_sb[:], channels=E,
                                       reduce_op=bass_isa.ReduceOp.max)
        eq = gw.tile([E, T], F32, tag="eq")
        nc.vector.tensor_tensor(eq[:], g_sb[:], m1b[:], op=OP.is_equal)
        gm = gw.tile([E, T], F32, tag="gm")
        nc.vector.scalar_tensor_tensor(gm[:], in0=eq[:], scalar=-1e30, in1=g_sb[:],
                                       op0=OP.mult, op1=OP.add)
        m2b = gw.tile([E, T], F32, tag="m2b")
        nc.gpsimd.partition_all_reduce(m2b[:], gm[:], channels=E,
                                       reduce_op=bass_isa.ReduceOp.max)
        # top-2 softmax weights: mask * sigmoid(2g - m1 - m2), scaled
        s12 = gw.tile([E, T], F32, tag="s12")
        nc.vector.tensor_tensor(s12[:], m1b[:], m2b[:], op=OP.add)
        arg = gw.tile([E, T], F32, tag="arg")
        nc.vector.scalar_tensor_tensor(arg[:], in0=g_sb[:], scalar=2.0, in1=s12[:],
                                       op0=OP.mult, op1=OP.subtract)
        sg = gw.tile([E, T], F32, tag="sg")
        nc.scalar.activation(sg[:], arg[:], mybir.ActivationFunctionType.Sigmoid)
        mask = gw.tile([E, T], F32, tag="mask")
        nc.vector.tensor_tensor(mask[:], g_sb[:], m2b[:], op=OP.is_ge)
        wsc = gw.tile([E, T], BF16, tag="wsc")
        nc.vector.scalar_tensor_tensor(wsc[:], in0=mask[:], scalar=SCALING, in1=sg[:],
                                       op0=OP.mult, op1=OP.mult)
        # stage low out of PSUM so the weighting can read wrep from PSUM
        low_sb = gw.tile([ER, T], F32, tag="lowsb")
        nc.vector.tensor_copy(low_sb[:], low_ps[:])

        def w_block(ot):
            if ot + 2 <= OTN - 1 and ot >= 2:
                w_dma(ot + 2, 0, nc.scalar)
                w_dma(ot + 2, 1, nc.scalar)
            pb[ot] = psB.tile([128, T], F32, tag="pb", name=f"pb{ot}")
            for k in range(KT):
                nc.tensor.matmul(pb[ot][:], lhsT=w_at(ot, k), rhs=x_at(k),
                                 start=(k == 0), stop=False)

        def finish(ot):
            nc.tensor.matmul(pb[ot][:], lhsT=bT_sb[:, ot * 128:(ot + 1) * 128],
                             rhs=lowT_sb[:], start=False, stop=True)
            o_sb = outp.tile([128, T], F32, tag="o", name=f"o{ot}")
            nc.vector.tensor_scalar(o_sb[:], pb[ot][:],
                                    scalar1=bias_sb[:, ot:ot + 1], scalar2=None,
                                    op0=OP.add)
            nc.sync.dma_start(out=out[:, ot, :], in_=o_sb[:])

        # ---- W blocks for ot3/ot4 run while the gating chain executes;
        # wrep/lowT land after ot4 with slack to spare ----
        w_block(3)
        w_block(4)
        wrep_ps = psA.tile([ER, T], F32, tag="gatewrep")
        nc.tensor.matmul(wrep_ps[:], lhsT=Rm_sb[:], rhs=wsc[:], start=True, stop=True)
        lowT_sb = gw.tile([ER, T], BF16, tag="lowT")
        nc.vector.tensor_tensor(lowT_sb[:], wrep_ps[:], low_sb[:], op=OP.mult)

        # ---- catch-up: two finishes per block until the lag is back to 1,
        # then steady state with a single trailing finish at the end ----
        w_block(5); finish(0); finish(1)
        w_block(6); finish(2); finish(3)
        w_block(7); finish(4); finish(5)
        w_block(8); finish(6); finish(7)
        w_block(9); finish(8)
        for ot in range(10, OTN):
            w_block(ot)
            finish(ot - 1)
        # last finish in halves: the second half's bias-add + store overlap
        # the first half's, shortening the post-last-matmul tail
        ot = OTN - 1
        nc.tensor.matmul(pb[ot][:], lhsT=bT_sb[:, ot * 128:(ot + 1) * 128],
                         rhs=lowT_sb[:], start=False, stop=True)
        o_sb = outp.tile([128, T], F32, tag="o", name=f"o{ot}")
        for h in range(2):
            hs = slice(h * (T // 2), (h + 1) * (T // 2))
            nc.vector.tensor_scalar(o_sb[:, hs], pb[ot][:, hs],
                                    scalar1=bias_sb[:, ot:ot + 1], scalar2=None,
                                    op0=OP.add)
            nc.sync.dma_start(out=out[:, ot, hs], in_=o_sb[:, hs])


def build_module(debug=False):
    nc = bacc.Bacc("TRN2", target_bir_lowering=False, debug=debug)
    xT = nc.dram_tensor("xT", [128, KT, T], BF16, kind="ExternalInput")
    wT = nc.dram_tensor("wT", [128, OTN, KT, 128], BF16, kind="ExternalInput")
    wT01 = nc.dram_tensor("wT01", [128, KT, NMERGE * 128], BF16,
                          kind="ExternalInput")
    cT = nc.dram_tensor("cT", [128, KT, E + ER], BF16, kind="ExternalInput")
    bT = nc.dram_tensor("bT", [ER, O], BF16, kind="ExternalInput")
    bias2 = nc.dram_tensor("bias2", [128, OTN], F32, kind="ExternalInput")
    Rm = nc.dram_tensor("Rm", [E, ER], BF16, kind="ExternalInput")
    out = nc.dram_tensor("out", [128, OTN, T], F32, kind="ExternalOutput")
    with tile.TileContext(nc) as tc:
        build_body(nc, tc, (xT, wT, wT01, cT, bT, bias2, Rm, out))
    nc.compile()
    return nc


def shard_inputs(x, gate_w, base_w, base_b, lora_A, lora_B):
    """FULL inputs -> list of 8 per-core input maps (host-side, free)."""
    x = np.asarray(x, dtype=np.float32)
    gate_w = np.asarray(gate_w, dtype=np.float32)
    base_w = np.asarray(base_w, dtype=np.float32)
    base_b = np.asarray(base_b, dtype=np.float32)
    lora_A = np.asarray(lora_A, dtype=np.float32)
    lora_B = np.asarray(lora_B, dtype=np.float32)

    xf = x.reshape(B * S, D)
    # replicated smalls; gate_w and lora_A fused into one streamed tensor
    gT = gate_w.T.reshape(KT, 128, E).transpose(1, 0, 2)
    A_flat = lora_A.reshape(ER, D)
    aT = A_flat.T.reshape(KT, 128, ER).transpose(1, 0, 2)
    cT = np.ascontiguousarray(
        np.concatenate([gT, aT], axis=2)).astype(NP_BF16)   # [128, KT, 136]
    B_flat = lora_B.transpose(0, 2, 1).reshape(ER, O)   # [er, o]
    bT = B_flat.astype(NP_BF16)
    Rm = np.repeat(np.eye(E, dtype=np.float32), R, axis=1).astype(NP_BF16)
    wTf = base_w.reshape(OTN, 128, KT, 128).transpose(3, 0, 2, 1)
    wT = np.ascontiguousarray(wTf).astype(NP_BF16)
    wT01 = np.ascontiguousarray(np.concatenate(
        [wTf[:, j] for j in range(NMERGE)], axis=2)).astype(NP_BF16)
    bias2 = np.ascontiguousarray(base_b.reshape(OTN, 128).T)

    in_maps = []
    for c in range(N_CORES):
        x_c = xf[c * T:(c + 1) * T]                         # [T, D]
        xTc = np.ascontiguousarray(
            x_c.T.reshape(KT, 128, T).transpose(1, 0, 2)).astype(NP_BF16)
        in_maps.append({"xT": xTc, "wT": wT, "wT01": wT01, "cT": cT,
                        "bT": bT, "bias2": bias2, "Rm": Rm})
    return in_maps


def gather_outputs(results):
    """list of 8 per-core result maps -> FULL output [B, S, O]."""
    full = np.empty((B * S, O), dtype=np.float32)
    for c in range(N_CORES):
        oc = results[c]["out"]                              # [128, OTN, T]
        full[c * T:(c + 1) * T, :] = oc.transpose(2, 1, 0).reshape(T, O)
    return full.reshape(B, S, O)


_NC_CACHE = {}


def _get_module():
    if "nc" not in _NC_CACHE:
        _NC_CACHE["nc"] = build_module()
    return _NC_CACHE["nc"]


def run_sharded(in_maps, **run_kwargs):
    nc = _get_module()
    return run_bass_kernel_spmd(nc, in_maps, list(range(N_CORES)), **run_kwargs)


def kernel(x, gate_w, base_w, base_b, lora_A, lora_B):
    in_maps = shard_inputs(x, gate_w, base_w, base_b, lora_A, lora_B)
    res = run_sharded(in_maps)
    return gather_outputs(res.results)


# revision 34
# speedup vs baseline: 1.0113x; 1.0113x over previous
"""Trainium2 Bass kernel for a LoRA-MoE layer (gate top-2 softmax routing +
dense base linear + per-expert low-rank adapters), SPMD across 8 NeuronCores.

Math (per token t):
    logits = x @ gate_w.T                      # [E]
    top-2 softmax over logits -> dense w[E] (0 for non-selected)
    out = x @ base_w.T + base_b
        + SCALING * sum_e w[e] * (x @ lora_A[e].T) @ lora_B[e].T

Key identities:
  - with w folded into the rank-space activations,
        lora_out = (low * w_rep) @ B_all.T,  low = x @ A_all.T  (A_all: [E*R, D])
  - the two top-2 softmax weights are sigmoid(2*logit - m1 - m2) where m1/m2
    are the top-2 logit values (sigmoid(m1-m2) and sigmoid(m2-m1)).

Sharding: pure data parallel - 8 token groups of 512 tokens; every core
computes all 4096 out features for its tokens.  All matmul operands are
bf16 (PSUM accumulation stays fp32): same 1 col/cycle PE throughput as
f32r but 2x lighter DMA and 4x faster LDWEIGHTS (fast weight load).

Layout per core (contraction dim on partitions):
    out.T[o, t] = sum_d W[o, d] * x.T[d, t]    (x.T moving, W tiles stationary)

Schedule: the first out-tile's k-loop also carries the lora-low and gate
matmuls (one pass over x while x streams in from HBM); each out-tile's
lora accumulate + store is deferred behind the next tile's W matmuls so
the gating vector chain never stalls the PE.
"""

import numpy as np
import ml_dtypes

import concourse.bass as bass
import concourse.bass_isa as bass_isa
import concourse.mybir as mybir
import concourse.tile as tile
from concourse import bacc
from concourse.bass_utils import run_bass_kernel_spmd

F32 = mybir.dt.float32
BF16 = mybir.dt.bfloat16
NP_BF16 = ml_dtypes.bfloat16

# Problem constants
B, S, D, O = 2, 2048, 4096, 4096
E, R = 8, 16
ER = E * R  # 128
SCALING = 32.0 / 16.0

# Sharding: 8 token groups (pure data parallel)
N_CORES = 8
T = (B * S) // N_CORES  # 512 tokens per core
KT = D // 128           # 32 contraction tiles
OTN = O // 128          # 32 out tiles per core
WCW = 16                # W DMA chunks per out tile: 2 chunks of 16 k-tiles
# k-tile chunk boundaries for the streamed first-pass tensors (x, c, w012):
# small quanta first so the PE can start early, then 4-k quanta (each dma
# issue costs ~0.63us on the issuing queue, so quanta can't be too small)
CB = [0, 2, 4, 8, 12, 16, 20, 24, 28, 32]
NCH = len(CB) - 1
NMERGE = 3              # out-tiles folded into the first k-loop (ot0..2)


def _ci(k):
    for i in range(NCH):
        if k < CB[i + 1]:
            return i
    raise ValueError(k)


def build_body(nc, tc, tensors):
    xT, wT, wT01, cT, bT, bias2, Rm, out = tensors
    OP = mybir.AluOpType

    with (
        tc.tile_pool(name="xp", bufs=NCH) as xp,
        tc.tile_pool(name="apl", bufs=NCH) as apl,
        tc.tile_pool(name="wp", bufs=8) as wp,
        tc.tile_pool(name="wp01", bufs=NCH) as wp01,
        tc.tile_pool(name="cst", bufs=1) as cst,
        tc.tile_pool(name="gw", bufs=1) as gw,
        tc.tile_pool(name="outp", bufs=3) as outp,
        tc.tile_pool(name="psA", bufs=1, space="PSUM") as psA,
        tc.tile_pool(name="psB", bufs=6, space="PSUM") as psB,
    ):
        # ---- tiny constants on the scalar HWDGE queue (it starves behind
        # the sync stream but these are only needed tens of us in); gpsimd
        # issues no DMAs at all so the tail SWDGE drain is empty ----
        Rm_sb = cst.tile([E, ER], BF16, tag="rm")
        nc.scalar.dma_start(out=Rm_sb[:], in_=Rm[:])
        bias_sb = cst.tile([128, OTN], F32, tag="bias")
        nc.scalar.dma_start(out=bias_sb[:], in_=bias2[:])
        bT_sb = cst.tile([ER, O], BF16, tag="bt")
        nc.scalar.dma_start(out=bT_sb[:], in_=bT[:])

        # preload the ACT sigmoid table off the gating chain's critical path
        sgdum = gw.tile([1, 8], F32, tag="sgdum")
        nc.scalar.activation(sgdum[:], bias_sb[0:1, 0:8],
                             mybir.ActivationFunctionType.Sigmoid)

        # NOTE on the PE clock gate (HAM): the PE starts at 1.2GHz and only
        # reaches 2.4GHz after ~3.4us of high-duty busy.  A pre-warm dummy
        # burst backfires here: the warmed PE then outruns the still-ramping
        # DMA stream, idles >3.4us, and gets re-throttled.  The cold-start
        # merged loop (432ns/matmul, 100% duty) matches the DMA ramp and
        # warms itself with no stalls.

        # ---- streaming inputs on sync, in exact consumption order.  The
        # merged loop runs 5 matmuls per k-tile (low, gate, W-ot0/1/2),
        # eating x(131K) + c(35K) + w012(98K) bf16 per 1.08us = ~245 GB/s,
        # with real slack against the ~330 GB/s a single HWDGE stream
        # delivers.  gate_w/lora_A are fused into one cT tensor and the
        # first three W tiles into one wT012 tensor so each quantum is only
        # three dma issues (~0.63us of issue time each on the sync queue) ----
        x_ch, c_ch, w012_ch = [], [], []
        w_ch = {}

        def w_dma(ot, j, eng):
            wq = wp.tile([128, WCW, 128], BF16, tag="w", name=f"w{ot}_{j}")
            eng.dma_start(out=wq[:], in_=wT[:, ot, j * WCW:(j + 1) * WCW, :])
            w_ch[(ot, j)] = wq

        for c in range(NCH):
            k0, k1 = CB[c], CB[c + 1]
            xc = xp.tile([128, k1 - k0, T], BF16, tag="x", name=f"x{c}")
            nc.sync.dma_start(out=xc[:], in_=xT[:, k0:k1, :])
            x_ch.append(xc)
            cc = apl.tile([128, k1 - k0, E + ER], BF16, tag="c", name=f"c{c}")
            nc.sync.dma_start(out=cc[:], in_=cT[:, k0:k1, :])
            c_ch.append(cc)
            wc = wp01.tile([128, k1 - k0, NMERGE * 128], BF16, tag="w012",
                           name=f"w012_{c}")
            nc.sync.dma_start(out=wc[:], in_=wT01[:, k0:k1, :])
            w012_ch.append(wc)
        w_dma(3, 0, nc.sync); w_dma(3, 1, nc.sync)
        w_dma(4, 0, nc.sync); w_dma(4, 1, nc.sync)

        def x_at(k):
            c = _ci(k)
            return x_ch[c][:, k - CB[c], :]

        def w_at(ot, k):
            if ot < NMERGE:
                c = _ci(k)
                return w012_ch[c][:, k - CB[c], ot * 128:(ot + 1) * 128]
            return w_ch[(ot, k // WCW)][:, k % WCW, :]

        # ---- merged first pass: low.T, gate.T and out-tiles 0-2 W matmuls ----
        low_ps = psA.tile([ER, T], F32, tag="low")
        gate_ps = psA.tile([E, T], F32, tag="gatewrep")
        pb = {}
        for ot in range(NMERGE):
            pb[ot] = psB.tile([128, T], F32, tag="pb", name=f"pb{ot}")
        # runs of same-accumulator matmuls per quantum: switching the PSUM
        # accumulation target costs ~90ns, so amortize it over 2-4 matmuls.
        # After the first two quanta, bridge the still-ramping DMA stream
        # with a few more dummy matmuls: a PE-idle run >3.4us re-throttles
        # the clock to 1.2GHz, which costs far more than the dummies
        for c in range(NCH):
            k0, k1 = CB[c], CB[c + 1]
            for k in range(k0, k1):
                nc.tensor.matmul(low_ps[:], lhsT=c_ch[c][:, k - k0, E:E + ER],
                                 rhs=x_at(k), start=(k == 0), stop=(k == KT - 1))
            for k in range(k0, k1):
                nc.tensor.matmul(gate_ps[:], lhsT=c_ch[c][:, k - k0, 0:E],
                                 rhs=x_at(k), start=(k == 0), stop=(k == KT - 1))
            for ot in range(NMERGE):
                for k in range(k0, k1):
                    nc.tensor.matmul(pb[ot][:], lhsT=w_at(ot, k), rhs=x_at(k),
                                     start=(k == 0), stop=False)


        # ---- gating math in [E, t] layout (DVE/ACT/GPSIMD, off PE path) ----
        g_sb = gw.tile([E, T], F32, tag="gsb")
        nc.vector.tensor_copy(g_sb[:], gate_ps[:])
        m1b = gw.tile([E, T], F32, tag="m1b")
        nc.gpsimd.partition_all_reduce(m1b[:], g_sb[:], channels=E,
                                       reduce_op=bass_isa.ReduceOp.max)
        eq = gw.tile([E, T], F32, tag="eq")
        nc.vector.tensor_tensor(eq[:], g_sb[:], m1b[:], op=OP.is_equal)
        gm = gw.tile([E, T], F32, tag="gm")
        nc.vector.scalar_tensor_tensor(gm[:], in0=eq[:], scalar=-1e30, in1=g_sb[:],
                                       op0=OP.mult, op1=OP.add)
        m2b = gw.tile([E, T], F32, tag="m2b")
        nc.gpsimd.partition_all_reduce(m2b[:], gm[:], channels=E,
                                       reduce_op=bass_isa.ReduceOp.max)
        # top-2 softmax weights: mask * sigmoid(2g - m1 - m2), scaled
        s12 = gw.tile([E, T], F32, tag="s12")
        nc.vector.tensor_tensor(s12[:], m1b[:], m2b[:], op=OP.add)
        arg = gw.tile([E, T], F32, tag="arg")
        nc.vector.scalar_tensor_tensor(arg[:], in0=g_sb[:], scalar=2.0, in1=s12[:],
                                       op0=OP.mult, op1=OP.subtract)
        sg = gw.tile([E, T], F32, tag="sg")
        nc.scalar.activation(sg[:], arg[:], mybir.ActivationFunctionType.Sigmoid)
        mask = gw.tile([E, T], F32, tag="mask")
        nc.vector.tensor_tensor(mask[:], g_sb[:], m2b[:], op=OP.is_ge)
        wsc = gw.tile([E, T], BF16, tag="wsc")
        nc.vector.scalar_tensor_tensor(wsc[:], in0=mask[:], scalar=SCALING, in1=sg[:],
                                       op0=OP.mult, op1=OP.mult)
        # stage low out of PSUM so the weighting can read wrep from PSUM
        low_sb = gw.tile([ER, T], F32, tag="lowsb")
        nc.vector.tensor_copy(low_sb[:], low_ps[:])

        def w_block(ot):
            if ot + 2 <= OTN - 1 and ot >= 2:
                w_dma(ot + 2, 0, nc.scalar)
                w_dma(ot + 2, 1, nc.scalar)
            pb[ot] = psB.tile([128, T], F32, tag="pb", name=f"pb{ot}")
            for k in range(KT):
                nc.tensor.matmul(pb[ot][:], lhsT=w_at(ot, k), rhs=x_at(k),
                                 start=(k == 0), stop=False)

        def finish(ot):
            nc.tensor.matmul(pb[ot][:], lhsT=bT_sb[:, ot * 128:(ot + 1) * 128],
                             rhs=lowT_sb[:], start=False, stop=True)
            o_sb = outp.tile([128, T], F32, tag="o", name=f"o{ot}")
            nc.vector.tensor_scalar(o_sb[:], pb[ot][:],
                                    scalar1=bias_sb[:, ot:ot + 1], scalar2=None,
                                    op0=OP.add)
            nc.sync.dma_start(out=out[:, ot, :], in_=o_sb[:])

        # ---- W blocks for ot3/ot4 run while the gating chain executes;
        # wrep/lowT land after ot4 with slack to spare ----
        w_block(3)
        w_block(4)
        wrep_ps = psA.tile([ER, T], F32, tag="gatewrep")
        nc.tensor.matmul(wrep_ps[:], lhsT=Rm_sb[:], rhs=wsc[:], start=True, stop=True)
        lowT_sb = gw.tile([ER, T], BF16, tag="lowT")
        nc.vector.tensor_tensor(lowT_sb[:], wrep_ps[:], low_sb[:], op=OP.mult)

        # ---- catch-up: two finishes per block until the lag is back to 1,
        # then steady state with a single trailing finish at the end ----
        w_block(5); finish(0); finish(1)
        w_block(6); finish(2); finish(3)
        w_block(7); finish(4); finish(5)
        w_block(8); finish(6); finish(7)
        w_block(9); finish(8)
        for ot in range(10, OTN):
            w_block(ot)
            finish(ot - 1)
        # last finish in halves: the second half's bias-add + store overlap
        # the first half's, shortening the post-last-matmul tail
        ot = OTN - 1
        nc.tensor.matmul(pb[ot][:], lhsT=bT_sb[:, ot * 128:(ot + 1) * 128],
                         rhs=lowT_sb[:], start=False, stop=True)
        o_sb = outp.tile([128, T], F32, tag="o", name=f"o{ot}")
        for h in range(2):
            hs = slice(h * (T // 2), (h + 1) * (T // 2))
            nc.vector.tensor_scalar(o_sb[:, hs], pb[ot][:, hs],
                                    scalar1=bias_sb[:, ot:ot + 1], scalar2=None,
                                    op0=OP.add)
            nc.sync.dma_start(out=out[:, ot, hs], in_=o_sb[:, hs])


def build_module(debug=False):
    nc = bacc.Bacc("TRN2", target_bir_lowering=False, debug=debug)
    xT = nc.dram_tensor("xT", [128, KT, T], BF16, kind="ExternalInput")
    wT = nc.dram_tensor("wT", [128, OTN, KT, 128], BF16, kind="ExternalInput")
    wT01 = nc.dram_tensor("wT01", [128, KT, NMERGE * 128], BF16,
                          kind="ExternalInput")
    cT = nc.dram_tensor("cT", [128, KT, E + ER], BF16, kind="ExternalInput")
    bT = nc.dram_tensor("bT", [ER, O], BF16, kind="ExternalInput")
    bias2 = nc.dram_tensor("bias2", [128, OTN], F32, kind="ExternalInput")
    Rm = nc.dram_tensor("Rm", [E, ER], BF16, kind="ExternalInput")
    out = nc.dram_tensor("out", [128, OTN, T], F32, kind="ExternalOutput")
    with tile.TileContext(nc) as tc:
        build_body(nc, tc, (xT, wT, wT01, cT, bT, bias2, Rm, out))
    nc.compile()
    return nc


def shard_inputs(x, gate_w, base_w, base_b, lora_A, lora_B):
    """FULL inputs -> list of 8 per-core input maps (host-side, free)."""
    x = np.asarray(x, dtype=np.float32)
    gate_w = np.asarray(gate_w, dtype=np.float32)
    base_w = np.asarray(base_w, dtype=np.float32)
    base_b = np.asarray(base_b, dtype=np.float32)
    lora_A = np.asarray(lora_A, dtype=np.float32)
    lora_B = np.asarray(lora_B, dtype=np.float32)

    xf = x.reshape(B * S, D)
    # replicated smalls; gate_w and lora_A fused into one streamed tensor
    gT = gate_w.T.reshape(KT, 128, E).transpose(1, 0, 2)
    A_flat = lora_A.reshape(ER, D)
    aT = A_flat.T.reshape(KT, 128, ER).transpose(1, 0, 2)
    cT = np.ascontiguousarray(
        np.concatenate([gT, aT], axis=2)).astype(NP_BF16)   # [128, KT, 136]
    B_flat = lora_B.transpose(0, 2, 1).reshape(ER, O)   # [er, o]
    bT = B_flat.astype(NP_BF16)
    Rm = np.repeat(np.eye(E, dtype=np.float32), R, axis=1).astype(NP_BF16)
    wTf = base_w.reshape(OTN, 128, KT, 128).transpose(3, 0, 2, 1)
    wT = np.ascontiguousarray(wTf).astype(NP_BF16)
    wT01 = np.ascontiguousarray(np.concatenate(
        [wTf[:, j] for j in range(NMERGE)], axis=2)).astype(NP_BF16)
    bias2 = np.ascontiguousarray(base_b.reshape(OTN, 128).T)

    in_maps = []
    for c in range(N_CORES):
        x_c = xf[c * T:(c + 1) * T]                         # [T, D]
        xTc = np.ascontiguousarray(
            x_c.T.reshape(KT, 128, T).transpose(1, 0, 2)).astype(NP_BF16)
        in_maps.append({"xT": xTc, "wT": wT, "wT01": wT01, "cT": cT,
                        "bT": bT, "bias2": bias2, "Rm": Rm})
    return in_maps


def gather_outputs(results):
    """list of 8 per-core result maps -> FULL output [B, S, O]."""
    full = np.empty((B * S, O), dtype=np.float32)
    for c in range(N_CORES):
        oc = results[c]["out"]                              # [128, OTN, T]
        full[c * T:(c + 1) * T, :] = oc.transpose(2, 1, 0).reshape(T, O)
    return full.reshape(B, S, O)


_NC_CACHE = {}


def _get_module():
    if "nc" not in _NC_CACHE:
        _NC_CACHE["nc"] = build_module()
    return _NC_CACHE["nc"]


def run_sharded(in_maps, **run_kwargs):
    nc = _get_module()
    return run_bass_kernel_spmd(nc, in_maps, list(range(N_CORES)), **run_kwargs)


def kernel(x, gate_w, base_w, base_b, lora_A, lora_B):
    in_maps = shard_inputs(x, gate_w, base_w, base_b, lora_A, lora_B)
    res = run_sharded(in_maps)
    return gather_outputs(res.results)
